# revision 19
# baseline (speedup 1.0000x reference)
"""Trainium2 Bass kernel for the depth-dependent camera rendering problem.

Strategy
--------
Host (numpy, float64): PSF synthesis (phase -> Hankel einsum -> radial
interp -> quadrant mirror -> fftshift -> normalize) and rfft2 of the PSF
(tiny: ~1% of FLOPs), plus input sharding.

Key algebraic identity (telescoping): with s_d = blur(cumsum_alpha)_d + eps,
  1 - ba_d = s_{d+1}/s_d,  over_d = prod_{d'<d}(1-ba_{d'}) = s_d/s_0,
  captimg = sum_d over_d * bv_d = (sum_d blur_d(vol_d)) / s_0,
  s_0 = sum_d blur_d(lay_d) + eps.
So the whole occlusion compositing reduces to two depth-summed blurs and one
final division.  Both sums accumulate in the FREQUENCY domain, so only TWO
inverse DFTs are needed in total (not per depth plane).

Device (Bass/Tile, 8 NeuronCores, SPMD): 6 cores each own one (b, c)
chain.  Per core, per depth plane d:
  lay_d = (idx == d); vol_d = lay_d * img/scale
  zv = DFT2(vol_d), zl = DFT2(lay_d)          (matmul-only, no transposes)
  FV += Fpsf_d * zv;  FA += Fpsf_d * zl       (elementwise complex MAC)
then  num = IDFT2(FV);  c0 = IDFT2(FA);  out = scale * num / (c0 + eps).

2D DFT via chained matmuls (out = lhsT.T @ rhs contracts the partition
axis and swaps the other two), so the plane layout ping-pongs and no
transposes are ever needed:
  [H, W] --tw1--> [W, hf(193)] --sym--> [W, hf(384)] --c2--> [hf, wf(193)]
  [hf, wf] --iA--> [wf, H] --iB--> [H, W]             (inverse)

The forward pass1 exploits real-input conjugate symmetry: only 193 of
384 hf bins are computed by matmul; the upper bins are reconstructed
with reversed-free-axis copies (y1[w, N-hf] = conj(y1[w, hf])), halving
pass1 tensor-engine time.

The complex MAC uses packed PSF tables pfa=[re|im], pfb=[im|-re] so each
chunk is 2 muls + 1 sub + 1 add of width 386 (instead of 8 ops of 193),
split across the DVE and Pool engines.
"""

import os
import time

import numpy as np

import concourse.bass as bass
import concourse.tile as tile
from concourse import bacc, mybir
from concourse.bass_utils import run_bass_kernel_spmd

dt = mybir.dt
Alu = mybir.AluOpType

# ---- problem constants (hardcoded; kernel.py must be self-contained) ----
N = 384            # image H = W
HF = N             # full spectrum bins along H
WF = N // 2 + 1    # rfft bins along W = 193
D = 16             # depth planes
B, C = 2, 3
EPS = 1e-3
NCORES = 8
WAVELENGTHS = np.array([632e-9, 550e-9, 450e-9])
FOCAL_LENGTH = 50e-3
FOCAL_DEPTH = 1.7
SENSOR_DIST = 1.0 / (1.0 / FOCAL_LENGTH - 1.0 / FOCAL_DEPTH)

MM_DT = dt.float32r   # matmul operand mode (full-rate); set dt.float32 for precision


# =====================================================================
# Host-side DFT tables
# =====================================================================
def _make_tables():
    k = np.arange(N, dtype=np.float64)
    th = 2.0 * np.pi * np.outer(k, k) / N     # [N, N]
    co = np.cos(th)
    sn = np.sin(th)
    c1 = np.concatenate([co, -sn], axis=1)                      # [N, 2N]
    tw1 = np.concatenate([co[:, :WF], -sn[:, :WF]], axis=1)     # [N, 2*WF]
    c2a = np.concatenate([co[:, :WF], -sn[:, :WF]], axis=1)     # [N, 2*WF]
    c2b = np.concatenate([sn[:, :WF], co[:, :WF]], axis=1)      # [N, 2*WF]
    b = np.full(WF, 2.0)
    b[0] = 1.0
    b[WF - 1] = 1.0
    ibr = b[:, None] * co[:WF, :]                               # [WF, N]
    ibi = -b[:, None] * sn[:WF, :]                              # [WF, N]
    ib = np.stack([ibr, ibi], axis=0)                           # [2, WF, N]
    return (c1.astype(np.float32), sn.astype(np.float32),
            tw1.astype(np.float32),
            c2a.astype(np.float32), c2b.astype(np.float32),
            ib.astype(np.float32))


# =====================================================================
# Device program
# =====================================================================
def build_program(occlusion: bool, n_depth: int = D):
    nc = bacc.Bacc(None, target_bir_lowering=False, debug=False)
    f32 = dt.float32

    img_d = nc.declare_dram_parameter("img", [N, N], f32, isOutput=False)
    idx_d = nc.declare_dram_parameter("idx", [N, N], f32, isOutput=False)
    pfc_d = nc.declare_dram_parameter("pfc", [D, HF, 4 * WF], f32, isOutput=False)
    c1_d = nc.declare_dram_parameter("c1", [N, 2 * N], f32, isOutput=False)
    si_d = nc.declare_dram_parameter("si", [N, N], f32, isOutput=False)
    tw1_d = nc.declare_dram_parameter("tw1", [N, 2 * WF], f32, isOutput=False)
    c2a_d = nc.declare_dram_parameter("c2a", [N, 2 * WF], f32, isOutput=False)
    c2b_d = nc.declare_dram_parameter("c2b", [N, 2 * WF], f32, isOutput=False)
    ib_d = nc.declare_dram_parameter("ib", [2, WF, N], f32, isOutput=False)
    out_d = nc.declare_dram_parameter("out", [N, N], f32, isOutput=True)

    PCH = [(0, 128), (128, 256), (256, 384)]          # partition chunks of 384
    WCH = [(0, 97), (97, WF)]                          # partition chunks of 193

    with tile.TileContext(nc) as tc:
        with (
            tc.tile_pool(name="const", bufs=1) as cp,
            tc.tile_pool(name="pers", bufs=1) as pp,
            tc.tile_pool(name="work", bufs=2) as wp,
            tc.tile_pool(name="y1p", bufs=2) as y1p,
            tc.tile_pool(name="pbp", bufs=2) as pbp,
            tc.tile_pool(name="pfp", bufs=2) as pfp,
            # "mm" rotates PSUM banks across pass1/pass2/stepB matmul groups
            tc.tile_pool(name="psmm", bufs=6, space="PSUM") as ps_mm,
            tc.tile_pool(name="pspr", bufs=1, space="PSUM") as ps_pr,
            tc.tile_pool(name="pspi", bufs=1, space="PSUM") as ps_pi,
        ):
            # ---- load constants ----
            def load3(dram, cols, tag, dtype=f32):
                ts = []
                for ci, (lo, hi) in enumerate(PCH):
                    t = cp.tile([128, cols], dtype, name=f"{tag}{ci}", tag=f"{tag}{ci}")
                    eng = nc.gpsimd if dtype != f32 else nc.sync
                    eng.dma_start(t[:], dram[lo:hi, :])
                    ts.append(t)
                return ts

            c1t = load3(c1_d, 2 * N, "c1", MM_DT)     # [cos | -sin] over [h, k]
            sit = load3(si_d, N, "si", MM_DT)         # sin
            tw1t = load3(tw1_d, 2 * WF, "tw1", MM_DT)
            c2at = load3(c2a_d, 2 * WF, "c2a", MM_DT)
            c2bt = load3(c2b_d, 2 * WF, "c2b", MM_DT)
            ibt = []                            # ib chunks: [2][wf-chunk]
            for comp in range(2):
                row = []
                for ci, (lo, hi) in enumerate(WCH):
                    t = cp.tile([hi - lo, N], MM_DT, name=f"ib{comp}{ci}", tag=f"ib{comp}{ci}")
                    nc.gpsimd.dma_start(t[:], ib_d[comp, lo:hi, :])
                    row.append(t)
                ibt.append(row)

            imgt = load3(img_d, N, "img")
            idxt = load3(idx_d, N, "idx")

            # persistent frequency accumulators [hf-chunk][128, re|im]
            FV = [pp.tile([128, 2 * WF], MM_DT, name=f"fv{ci}", tag=f"fv{ci}") for ci in range(3)]
            if occlusion:
                FA = [pp.tile([128, 2 * WF], MM_DT, name=f"fa{ci}", tag=f"fa{ci}") for ci in range(3)]

            # ---------------- helpers ----------------
            def fwd_p(x3, name):
                """x3: 3 tiles [128, N] ([H, W]) -> 3 PSUM tiles [128, 2*WF]
                (layout [hf, (re|im)]), left in PSUM for the accumulator."""
                y1 = [y1p.tile([128, 2 * N], MM_DT, name=f"y1_{m}", tag=f"y1_{m}") for m in range(3)]
                for m in range(3):
                    p1 = ps_mm.tile([128, 2 * WF], f32, name="p1", tag="mm")
                    for k in range(3):
                        nc.tensor.matmul(
                            p1[:], x3[k][:, m * 128:(m + 1) * 128],
                            tw1t[k][:],
                            start=(k == 0), stop=(k == 2))
                    # straight blocks: one op via strided 3D APs
                    nc.scalar.copy(
                        y1[m].rearrange("p (a x) -> p a x", a=2)[:, :, 0:WF],
                        p1.rearrange("p (a x) -> p a x", a=2)[:, :, :])
                    # conjugate mirror: Yr[hf] = y1r[N-hf], Yi[hf] = -y1i[N-hf]
                    nc.scalar.copy(y1[m][:, WF:N], p1[:, WF - 2:0:-1])
                    nc.scalar.mul(y1[m][:, N + WF:2 * N], p1[:, 2 * WF - 2:WF:-1], -1.0)
                pzs = []
                for m in range(3):
                    pz = ps_mm.tile([128, 2 * WF], f32, name=f"pz_{name}{m}", tag="mm")
                    for k in range(3):
                        nc.tensor.matmul(
                            pz[:], y1[k][:, m * 128:(m + 1) * 128],
                            c2at[k][:],
                            start=(k == 0), stop=False)
                        nc.tensor.matmul(
                            pz[:], y1[k][:, N + m * 128:N + (m + 1) * 128],
                            c2bt[k][:],
                            start=False, stop=(k == 2))
                    pzs.append(pz)
                return pzs

            def accum(pzs, pfct, FX, first, name):
                """FX[ci] += Fpsf_d * z   (complex, elementwise).
                U = [z|z] * [pfa|pfb] = [zr*pr, zi*pi, zr*pi, -zi*pr];
                w = U[0]-U[1] | U[2]-U[3] = [Re | Im] of z*P.
                GPSIMD cannot touch PSUM, so the mul (reads pz in PSUM) is on
                DVE; sub/add (SBUF-only) go to Pool."""
                for ci in range(3):
                    U = wp.tile([128, 4 * WF], f32, name=f"u_{name}{ci}", tag=f"u{ci}")
                    zb = pzs[ci][:].unsqueeze(1).broadcast_to([128, 2, 2 * WF])
                    nc.vector.tensor_mul(
                        U.rearrange("p (a x) -> p a x", a=2), zb,
                        pfct[ci].rearrange("p (a x) -> p a x", a=2))
                    U4 = U.rearrange("p (a x) -> p a x", a=4)
                    if first:
                        nc.gpsimd.tensor_sub(
                            FX[ci].rearrange("p (a x) -> p a x", a=2),
                            U4[:, 0::2, :], U4[:, 1::2, :])
                    else:
                        w = wp.tile([128, 2 * WF], f32, name=f"w_{name}{ci}", tag=f"w{ci}")
                        nc.gpsimd.tensor_sub(
                            w.rearrange("p (a x) -> p a x", a=2),
                            U4[:, 0::2, :], U4[:, 1::2, :])
                        nc.vector.tensor_add(FX[ci][:], FX[ci][:], w[:])

            def inv(f3, name, dst_pool, dst_tag):
                """f3: 3 tiles [128, 2*WF] -> y: 3 tiles [128, N] (layout [H, W])."""
                pch = []
                for mi, (lo, hi) in enumerate(WCH):
                    w = hi - lo
                    t = pbp.tile([w, 2 * N], MM_DT, name=f"p_{mi}", tag=f"p_{mi}")
                    prr = ps_pr.tile([97, N], f32, name="ppr", tag="ppr")
                    pii = ps_pi.tile([97, N], f32, name="ppi", tag="ppi")
                    for k in range(3):
                        # Pr = Zr.T @ cos + Zi.T @ (-sin)
                        nc.tensor.matmul(
                            prr[:w], f3[k][:, lo:hi],
                            c1t[k][:, 0:N],
                            start=(k == 0), stop=False)
                        nc.tensor.matmul(
                            prr[:w], f3[k][:, WF + lo:WF + hi],
                            c1t[k][:, N:2 * N],
                            start=False, stop=(k == 2))
                        # Pi = Zr.T @ sin + Zi.T @ cos
                        nc.tensor.matmul(
                            pii[:w], f3[k][:, lo:hi],
                            sit[k][:],
                            start=(k == 0), stop=False)
                        nc.tensor.matmul(
                            pii[:w], f3[k][:, WF + lo:WF + hi],
                            c1t[k][:, 0:N],
                            start=False, stop=(k == 2))
                    nc.any.tensor_copy(t[:, 0:N], prr[:w])
                    nc.any.tensor_copy(t[:, N:2 * N], pii[:w])
                    pch.append(t)
                y = [dst_pool.tile([128, N], f32, name=f"{dst_tag}{m}", tag=f"{dst_tag}{m}") for m in range(3)]
                for m in range(3):
                    py = ps_mm.tile([128, N], f32, name="py", tag="mm")
                    for k, (lo, hi) in enumerate(WCH):
                        w = hi - lo
                        nc.tensor.matmul(
                            py[:], pch[k][:w, m * 128:(m + 1) * 128],
                            ibt[0][k][:],
                            start=(k == 0), stop=False)
                        nc.tensor.matmul(
                            py[:], pch[k][:w, N + m * 128:N + (m + 1) * 128],
                            ibt[1][k][:],
                            start=False, stop=(k == 1))
                    nc.any.tensor_copy(y[m][:], py[:])
                return y

            # ---------------- main depth loop (software-pipelined) ----------------
            def load_pf(dd):
                pfct = []
                for ci, (lo, hi) in enumerate(PCH):
                    ta = pfp.tile([128, 4 * WF], f32, name=f"pfc{ci}", tag=f"pfc{ci}")
                    nc.sync.dma_start(ta[:], pfc_d[dd, lo:hi, :])
                    pfct.append(ta)
                return pfct

            def make_layvol(dd):
                lay = [wp.tile([128, N], MM_DT, name=f"lay{ci}", tag=f"lay{ci}") for ci in range(3)]
                vol = [wp.tile([128, N], MM_DT, name=f"vol{ci}", tag=f"vol{ci}") for ci in range(3)]
                for ci in range(3):
                    nc.gpsimd.tensor_scalar(
                        lay[ci][:], idxt[ci][:], float(dd), None, op0=Alu.is_equal)
                    nc.gpsimd.tensor_mul(vol[ci][:], lay[ci][:], imgt[ci][:])
                return lay, vol

            pf_cur = load_pf(0)
            lay, vol = make_layvol(0)
            for dd in range(n_depth):
                first = (dd == 0)
                pzv = fwd_p(vol, f"v{dd}")
                # emit next plane's inputs here: DVE/Pool fill idle while the
                # tensor engine runs this plane's transforms
                if dd + 1 < n_depth:
                    nxt = make_layvol(dd + 1)
                    pf_nxt = load_pf(dd + 1)
                accum(pzv, pf_cur, FV, first, "v")
                if occlusion:
                    pzl = fwd_p(lay, f"l{dd}")
                    accum(pzl, pf_cur, FA, first, "l")
                if dd + 1 < n_depth:
                    lay, vol = nxt
                    pf_cur = pf_nxt

            # ---------------- final inverses + division ----------------
            num = inv(FV, "nv", wp, "num")
            if occlusion:
                c0 = inv(FA, "na", wp, "cc")
                for ci in range(3):
                    rc = wp.tile([128, N], f32, name="rc", tag="rc")
                    nc.vector.tensor_scalar_add(rc[:], c0[ci][:], EPS)
                    nc.vector.reciprocal(rc[:], rc[:])
                    nc.gpsimd.tensor_mul(num[ci][:], num[ci][:], rc[:])

            # store
            for ci, (lo, hi) in enumerate(PCH):
                nc.sync.dma_start(out_d[lo:hi, :], num[ci][:])

    nc.compile()
    return nc


# =====================================================================
# Host-side PSF pipeline (float64, mirrors reference.py exactly)
# =====================================================================
def _host_psf(heightmap1d, prop_amplitude, prop_phase, H, rho_grid, rho_sampling):
    wl = WAVELENGTHS.reshape(3, 1, 1)
    hm = np.asarray(heightmap1d, np.float64).reshape(1, 1, -1)
    pa = np.asarray(prop_amplitude, np.float64)
    pp_ = np.asarray(prop_phase, np.float64)
    Hm = np.asarray(H, np.float64)
    rg = np.asarray(rho_grid, np.float64)
    rs = np.asarray(rho_sampling, np.float64)

    n_idx = 1.5375 + 0.00829045 / (wl * 1e6) ** 2 - 0.000211046 / (wl * 1e6) ** 4
    phase = 2.0 * np.pi / wl * (n_idx - 1.0) * hm + pp_          # [3,D,M]
    real = np.einsum('wdm,wmr->wdr', pa * np.cos(phase), Hm)
    imag = np.einsum('wdm,wmr->wdr', pa * np.sin(phase), Hm)
    psf1d = (2.0 * np.pi / (wl * SENSOR_DIST)) ** 2 * (real ** 2 + imag ** 2)

    hh = N // 2
    nd = psf1d.shape[1]
    psf_rd = np.empty((3, nd, hh * hh), np.float64)
    for w in range(3):
        sflat = rs[w].reshape(-1)
        for d in range(nd):
            psf_rd[w, d] = np.interp(sflat, rg[w], psf1d[w, d])
    psf_rd = np.maximum(psf_rd, 0.0).astype(np.float32).reshape(3, nd, hh, hh)
    q = np.concatenate([psf_rd[:, :, ::-1, :], psf_rd], axis=-2)
    psf = np.concatenate([q[:, :, :, ::-1], q], axis=-1)          # [3,D,N,N]
    psf = np.fft.fftshift(psf, axes=(-2, -1))
    psf = psf / np.sum(psf, axis=(-2, -1), keepdims=True)
    Fpsf = np.fft.rfft2(psf.astype(np.float64)) / float(N * N)    # [3,D,N,WF]
    re = Fpsf.real.astype(np.float32)
    im = Fpsf.imag.astype(np.float32)
    pfc = np.concatenate([re, im, im, -re], axis=-1)              # [3,D,N,4WF]
    return pfc


_PROG_CACHE = {}


def kernel(img, depthmap, heightmap1d, prop_amplitude, prop_phase, H,
           rho_grid, rho_sampling, occlusion):
    occ = bool(np.asarray(occlusion).item())
    img = np.asarray(img, np.float32)
    depthmap = np.asarray(depthmap, np.float32)

    pfc = _host_psf(heightmap1d, prop_amplitude, prop_phase, H,
                    rho_grid, rho_sampling)

    scale = np.float32(img.max())
    imgs = img / scale                                            # [B,C,N,N] f32
    idxf = np.clip(np.floor(depthmap * np.float32(D)), 0, D - 1)[:, 0]  # [B,N,N]
    c1, si, tw1, c2a, c2b, ib = _make_tables()

    if occ not in _PROG_CACHE:
        _PROG_CACHE[occ] = build_program(occ)
    nc = _PROG_CACHE[occ]

    in_maps = []
    for core in range(NCORES):
        b_, c_ = divmod(core, C) if core < B * C else (0, 0)
        in_maps.append({
            "img": np.ascontiguousarray(imgs[b_, c_]),
            "idx": np.ascontiguousarray(idxf[b_]),
            "pfc": np.ascontiguousarray(pfc[c_]),
            "c1": c1, "si": si, "tw1": tw1, "c2a": c2a, "c2b": c2b, "ib": ib,
        })
    t0 = time.perf_counter()
    res_obj = run_bass_kernel_spmd(
        nc, in_maps, list(range(NCORES)),
        trace=bool(os.environ.get("KBASS_TRACE")))
    global LAST
    LAST = {"wall_s": time.perf_counter() - t0,
            "exec_time_ns": res_obj.exec_time_ns,
            "profile_json": res_obj.profile_json}
    res = res_obj.results
    out = np.empty((B, C, N, N), np.float32)
    for core in range(B * C):
        b_, c_ = divmod(core, C)
        out[b_, c_] = res[core]["out"] * scale
    return out


# revision 22
# speedup vs baseline: 1.1686x; 1.1686x over previous
"""Trainium2 Bass kernel for the depth-dependent camera rendering problem.

Strategy
--------
Host (numpy, float64): PSF synthesis (phase -> Hankel einsum -> radial
interp -> quadrant mirror -> fftshift -> normalize) and rfft2 of the PSF
(tiny: ~1% of FLOPs), plus input sharding.

Key algebraic identity (telescoping): with s_d = blur(cumsum_alpha)_d + eps,
  1 - ba_d = s_{d+1}/s_d,  over_d = prod_{d'<d}(1-ba_{d'}) = s_d/s_0,
  captimg = sum_d over_d * bv_d = (sum_d blur_d(vol_d)) / s_0,
  s_0 = sum_d blur_d(lay_d) + eps.
So the whole occlusion compositing reduces to two depth-summed blurs and one
final division.  Both sums accumulate in the FREQUENCY domain, so only TWO
inverse DFTs are needed in total (not per depth plane).

Device (Bass/Tile, 8 NeuronCores, SPMD): 6 cores each own one (b, c)
chain.  Per core, per depth plane d:
  lay_d = (idx == d); vol_d = lay_d * img/scale
  zv = DFT2(vol_d), zl = DFT2(lay_d)          (matmul-only, no transposes)
  FV += Fpsf_d * zv;  FA += Fpsf_d * zl       (elementwise complex MAC)
then  num = IDFT2(FV);  c0 = IDFT2(FA);  out = scale * num / (c0 + eps).

2D DFT via chained matmuls (out = lhsT.T @ rhs contracts the partition
axis and swaps the other two), so the plane layout ping-pongs and no
transposes are ever needed:
  [H, W] --tw1--> [W, hf(193)] --sym--> [W, hf(384)] --c2--> [hf, wf(193)]
  [hf, wf] --iA--> [wf, H] --iB--> [H, W]             (inverse)

The forward pass1 exploits real-input conjugate symmetry: only 193 of
384 hf bins are computed by matmul; the upper bins are reconstructed
with reversed-free-axis copies (y1[w, N-hf] = conj(y1[w, hf])), halving
pass1 tensor-engine time.

The complex MAC uses packed PSF tables pfa=[re|im], pfb=[im|-re] so each
chunk is 2 muls + 1 sub + 1 add of width 386 (instead of 8 ops of 193),
split across the DVE and Pool engines.
"""

import os
import time

import numpy as np

import concourse.bass as bass
import concourse.tile as tile
from concourse import bacc, mybir
from concourse.bass_utils import run_bass_kernel_spmd

dt = mybir.dt
Alu = mybir.AluOpType

# ---- problem constants (hardcoded; kernel.py must be self-contained) ----
N = 384            # image H = W
HF = N             # full spectrum bins along H
WF = N // 2 + 1    # rfft bins along W = 193
D = 16             # depth planes
B, C = 2, 3
EPS = 1e-3
NCORES = 8
WAVELENGTHS = np.array([632e-9, 550e-9, 450e-9])
FOCAL_LENGTH = 50e-3
FOCAL_DEPTH = 1.7
SENSOR_DIST = 1.0 / (1.0 / FOCAL_LENGTH - 1.0 / FOCAL_DEPTH)

MM_DT = dt.float32r   # matmul operand mode (full-rate); set dt.float32 for precision


# =====================================================================
# Host-side DFT tables
# =====================================================================
def _make_tables():
    k = np.arange(N, dtype=np.float64)
    th = 2.0 * np.pi * np.outer(k, k) / N     # [N, N]
    co = np.cos(th)
    sn = np.sin(th)
    c1 = np.concatenate([co, -sn], axis=1)                      # [N, 2N]
    tw1 = np.concatenate([co[:, :WF], -sn[:, :WF]], axis=1)     # [N, 2*WF]
    c2a = np.concatenate([co[:, :WF], -sn[:, :WF]], axis=1)     # [N, 2*WF]
    c2b = np.concatenate([sn[:, :WF], co[:, :WF]], axis=1)      # [N, 2*WF]
    b = np.full(WF, 2.0)
    b[0] = 1.0
    b[WF - 1] = 1.0
    ibr = b[:, None] * co[:WF, :]                               # [WF, N]
    ibi = -b[:, None] * sn[:WF, :]                              # [WF, N]
    ib = np.stack([ibr, ibi], axis=0)                           # [2, WF, N]
    return (c1.astype(np.float32), sn.astype(np.float32),
            tw1.astype(np.float32),
            c2a.astype(np.float32), c2b.astype(np.float32),
            ib.astype(np.float32))


# =====================================================================
# Device program
# =====================================================================
def build_program(occlusion: bool, n_depth: int = D):
    nc = bacc.Bacc(None, target_bir_lowering=False, debug=False)
    f32 = dt.float32

    img_d = nc.declare_dram_parameter("img", [N, N], f32, isOutput=False)
    idx_d = nc.declare_dram_parameter("idx", [N, N], f32, isOutput=False)
    pfc_d = nc.declare_dram_parameter("pfc", [D, HF, 4 * WF], f32, isOutput=False)
    c1_d = nc.declare_dram_parameter("c1", [N, 2 * N], f32, isOutput=False)
    si_d = nc.declare_dram_parameter("si", [N, N], f32, isOutput=False)
    tw1_d = nc.declare_dram_parameter("tw1", [N, 2 * WF], f32, isOutput=False)
    c2a_d = nc.declare_dram_parameter("c2a", [N, 2 * WF], f32, isOutput=False)
    c2b_d = nc.declare_dram_parameter("c2b", [N, 2 * WF], f32, isOutput=False)
    ib_d = nc.declare_dram_parameter("ib", [2, WF, N], f32, isOutput=False)
    out_d = nc.declare_dram_parameter("out", [N, N], f32, isOutput=True)

    PCH = [(0, 128), (128, 256), (256, 384)]          # partition chunks of 384
    WCH = [(0, 97), (97, WF)]                          # partition chunks of 193

    with tile.TileContext(nc) as tc:
        with (
            tc.tile_pool(name="const", bufs=1) as cp,
            tc.tile_pool(name="pers", bufs=1) as pp,
            tc.tile_pool(name="work", bufs=2) as wp,
            tc.tile_pool(name="y1p", bufs=2) as y1p,
            tc.tile_pool(name="pbp", bufs=2) as pbp,
            tc.tile_pool(name="pfp", bufs=2) as pfp,
            # "mm" rotates PSUM banks across pass1/pass2/stepB matmul groups
            tc.tile_pool(name="psmm", bufs=6, space="PSUM") as ps_mm,
            tc.tile_pool(name="pspr", bufs=1, space="PSUM") as ps_pr,
            tc.tile_pool(name="pspi", bufs=1, space="PSUM") as ps_pi,
        ):
            # ---- load constants ----
            def load3(dram, cols, tag, dtype=f32):
                ts = []
                for ci, (lo, hi) in enumerate(PCH):
                    t = cp.tile([128, cols], dtype, name=f"{tag}{ci}", tag=f"{tag}{ci}")
                    eng = nc.gpsimd if dtype != f32 else nc.sync
                    eng.dma_start(t[:], dram[lo:hi, :])
                    ts.append(t)
                return ts

            c1t = load3(c1_d, 2 * N, "c1", MM_DT)     # [cos | -sin] over [h, k]
            sit = load3(si_d, N, "si", MM_DT)         # sin
            tw1t = load3(tw1_d, 2 * WF, "tw1", MM_DT)
            c2at = load3(c2a_d, 2 * WF, "c2a", MM_DT)
            c2bt = load3(c2b_d, 2 * WF, "c2b", MM_DT)
            ibt = []                            # ib chunks: [2][wf-chunk]
            for comp in range(2):
                row = []
                for ci, (lo, hi) in enumerate(WCH):
                    t = cp.tile([hi - lo, N], MM_DT, name=f"ib{comp}{ci}", tag=f"ib{comp}{ci}")
                    nc.gpsimd.dma_start(t[:], ib_d[comp, lo:hi, :])
                    row.append(t)
                ibt.append(row)

            imgt = load3(img_d, N, "img")
            idxt = load3(idx_d, N, "idx")

            # persistent frequency accumulators [hf-chunk][128, re|im]
            FV = [pp.tile([128, 2 * WF], MM_DT, name=f"fv{ci}", tag=f"fv{ci}") for ci in range(3)]
            if occlusion:
                FA = [pp.tile([128, 2 * WF], MM_DT, name=f"fa{ci}", tag=f"fa{ci}") for ci in range(3)]

            # ---------------- helpers ----------------
            def fwd_p(x3, name):
                """x3: 3 tiles [128, N] ([H, W]) -> 3 PSUM tiles [128, 2*WF]
                (layout [hf, (re|im)]), left in PSUM for the accumulator."""
                y1 = [y1p.tile([128, 2 * N], MM_DT, name=f"y1_{m}", tag=f"y1_{m}") for m in range(3)]
                for m in range(3):
                    p1 = ps_mm.tile([128, 2 * WF], f32, name="p1", tag="mm")
                    for k in range(3):
                        nc.tensor.matmul(
                            p1[:], x3[k][:, m * 128:(m + 1) * 128],
                            tw1t[k][:],
                            start=(k == 0), stop=(k == 2))
                    # straight blocks: one op via strided 3D APs
                    nc.scalar.copy(
                        y1[m].rearrange("p (a x) -> p a x", a=2)[:, :, 0:WF],
                        p1.rearrange("p (a x) -> p a x", a=2)[:, :, :])
                    # conjugate mirror: Yr[hf] = y1r[N-hf], Yi[hf] = -y1i[N-hf]
                    nc.scalar.copy(y1[m][:, WF:N], p1[:, WF - 2:0:-1])
                    nc.scalar.mul(y1[m][:, N + WF:2 * N], p1[:, 2 * WF - 2:WF:-1], -1.0)
                pzs = []
                for m in range(3):
                    pz = ps_mm.tile([128, 2 * WF], f32, name=f"pz_{name}{m}", tag="mm")
                    for k in range(3):
                        nc.tensor.matmul(
                            pz[:], y1[k][:, m * 128:(m + 1) * 128],
                            c2at[k][:],
                            start=(k == 0), stop=False)
                        nc.tensor.matmul(
                            pz[:], y1[k][:, N + m * 128:N + (m + 1) * 128],
                            c2bt[k][:],
                            start=False, stop=(k == 2))
                    pzs.append(pz)
                return pzs

            def accum(pzs, pfct, FX, first, name):
                """FX[ci] += Fpsf_d * z   (complex, elementwise).
                U = [z|z] * [pfa|pfb] = [zr*pr, zi*pi, zr*pi, -zi*pr];
                w = U[0]-U[1] | U[2]-U[3] = [Re | Im] of z*P.
                GPSIMD cannot touch PSUM, so the mul (reads pz in PSUM) is on
                DVE; sub/add (SBUF-only) go to Pool."""
                for ci in range(3):
                    U = wp.tile([128, 4 * WF], f32, name=f"u_{name}{ci}", tag=f"u{ci}")
                    zb = pzs[ci][:].unsqueeze(1).broadcast_to([128, 2, 2 * WF])
                    nc.vector.tensor_mul(
                        U.rearrange("p (a x) -> p a x", a=2), zb,
                        pfct[ci].rearrange("p (a x) -> p a x", a=2))
                    U4 = U.rearrange("p (a x) -> p a x", a=4)
                    if first:
                        nc.gpsimd.tensor_sub(
                            FX[ci].rearrange("p (a x) -> p a x", a=2),
                            U4[:, 0::2, :], U4[:, 1::2, :])
                    else:
                        w = wp.tile([128, 2 * WF], f32, name=f"w_{name}{ci}", tag=f"w{ci}")
                        nc.gpsimd.tensor_sub(
                            w.rearrange("p (a x) -> p a x", a=2),
                            U4[:, 0::2, :], U4[:, 1::2, :])
                        eadd = nc.vector if ci == 0 else nc.gpsimd
                        eadd.tensor_add(FX[ci][:], FX[ci][:], w[:])

            def inv(f3, name, dst_pool, dst_tag):
                """f3: 3 tiles [128, 2*WF] -> y: 3 tiles [128, N] (layout [H, W])."""
                pch = []
                for mi, (lo, hi) in enumerate(WCH):
                    w = hi - lo
                    t = pbp.tile([w, 2 * N], MM_DT, name=f"p_{mi}", tag=f"p_{mi}")
                    prr = ps_pr.tile([97, N], f32, name="ppr", tag="ppr")
                    pii = ps_pi.tile([97, N], f32, name="ppi", tag="ppi")
                    for k in range(3):
                        # Pr = Zr.T @ cos + Zi.T @ (-sin)
                        nc.tensor.matmul(
                            prr[:w], f3[k][:, lo:hi],
                            c1t[k][:, 0:N],
                            start=(k == 0), stop=False)
                        nc.tensor.matmul(
                            prr[:w], f3[k][:, WF + lo:WF + hi],
                            c1t[k][:, N:2 * N],
                            start=False, stop=(k == 2))
                        # Pi = Zr.T @ sin + Zi.T @ cos
                        nc.tensor.matmul(
                            pii[:w], f3[k][:, lo:hi],
                            sit[k][:],
                            start=(k == 0), stop=False)
                        nc.tensor.matmul(
                            pii[:w], f3[k][:, WF + lo:WF + hi],
                            c1t[k][:, 0:N],
                            start=False, stop=(k == 2))
                    nc.any.tensor_copy(t[:, 0:N], prr[:w])
                    nc.any.tensor_copy(t[:, N:2 * N], pii[:w])
                    pch.append(t)
                y = [dst_pool.tile([128, N], f32, name=f"{dst_tag}{m}", tag=f"{dst_tag}{m}") for m in range(3)]
                for m in range(3):
                    py = ps_mm.tile([128, N], f32, name="py", tag="mm")
                    for k, (lo, hi) in enumerate(WCH):
                        w = hi - lo
                        nc.tensor.matmul(
                            py[:], pch[k][:w, m * 128:(m + 1) * 128],
                            ibt[0][k][:],
                            start=(k == 0), stop=False)
                        nc.tensor.matmul(
                            py[:], pch[k][:w, N + m * 128:N + (m + 1) * 128],
                            ibt[1][k][:],
                            start=False, stop=(k == 1))
                    nc.any.tensor_copy(y[m][:], py[:])
                return y

            # ---------------- main depth loop (software-pipelined) ----------------
            def load_pf(dd):
                pfct = []
                for ci, (lo, hi) in enumerate(PCH):
                    ta = pfp.tile([128, 4 * WF], f32, name=f"pfc{ci}", tag=f"pfc{ci}")
                    nc.sync.dma_start(ta[:], pfc_d[dd, lo:hi, :])
                    pfct.append(ta)
                return pfct

            def make_layvol(dd):
                lay = [wp.tile([128, N], MM_DT, name=f"lay{ci}", tag=f"lay{ci}") for ci in range(3)]
                vol = [wp.tile([128, N], MM_DT, name=f"vol{ci}", tag=f"vol{ci}") for ci in range(3)]
                for ci in range(3):
                    nc.vector.tensor_scalar(
                        lay[ci][:], idxt[ci][:], float(dd), None, op0=Alu.is_equal)
                    nc.vector.tensor_mul(vol[ci][:], lay[ci][:], imgt[ci][:])
                return lay, vol

            pf_cur = load_pf(0)
            lay, vol = make_layvol(0)
            for dd in range(n_depth):
                first = (dd == 0)
                pzv = fwd_p(vol, f"v{dd}")
                # emit next plane's inputs here: DVE/Pool fill idle while the
                # tensor engine runs this plane's transforms
                if dd + 1 < n_depth:
                    nxt = make_layvol(dd + 1)
                    pf_nxt = load_pf(dd + 1)
                accum(pzv, pf_cur, FV, first, "v")
                if occlusion:
                    pzl = fwd_p(lay, f"l{dd}")
                    accum(pzl, pf_cur, FA, first, "l")
                if dd + 1 < n_depth:
                    lay, vol = nxt
                    pf_cur = pf_nxt

            # ---------------- final inverses + division ----------------
            num = inv(FV, "nv", wp, "num")
            if occlusion:
                c0 = inv(FA, "na", wp, "cc")
                for ci in range(3):
                    rc = wp.tile([128, N], f32, name="rc", tag="rc")
                    nc.vector.tensor_scalar_add(rc[:], c0[ci][:], EPS)
                    nc.vector.reciprocal(rc[:], rc[:])
                    nc.gpsimd.tensor_mul(num[ci][:], num[ci][:], rc[:])

            # store
            for ci, (lo, hi) in enumerate(PCH):
                nc.sync.dma_start(out_d[lo:hi, :], num[ci][:])

    nc.compile()
    return nc


# =====================================================================
# Host-side PSF pipeline (float64, mirrors reference.py exactly)
# =====================================================================
def _host_psf(heightmap1d, prop_amplitude, prop_phase, H, rho_grid, rho_sampling):
    wl = WAVELENGTHS.reshape(3, 1, 1)
    hm = np.asarray(heightmap1d, np.float64).reshape(1, 1, -1)
    pa = np.asarray(prop_amplitude, np.float64)
    pp_ = np.asarray(prop_phase, np.float64)
    Hm = np.asarray(H, np.float64)
    rg = np.asarray(rho_grid, np.float64)
    rs = np.asarray(rho_sampling, np.float64)

    n_idx = 1.5375 + 0.00829045 / (wl * 1e6) ** 2 - 0.000211046 / (wl * 1e6) ** 4
    phase = 2.0 * np.pi / wl * (n_idx - 1.0) * hm + pp_          # [3,D,M]
    real = np.einsum('wdm,wmr->wdr', pa * np.cos(phase), Hm)
    imag = np.einsum('wdm,wmr->wdr', pa * np.sin(phase), Hm)
    psf1d = (2.0 * np.pi / (wl * SENSOR_DIST)) ** 2 * (real ** 2 + imag ** 2)

    hh = N // 2
    nd = psf1d.shape[1]
    psf_rd = np.empty((3, nd, hh * hh), np.float64)
    for w in range(3):
        sflat = rs[w].reshape(-1)
        for d in range(nd):
            psf_rd[w, d] = np.interp(sflat, rg[w], psf1d[w, d])
    psf_rd = np.maximum(psf_rd, 0.0).astype(np.float32).reshape(3, nd, hh, hh)
    q = np.concatenate([psf_rd[:, :, ::-1, :], psf_rd], axis=-2)
    psf = np.concatenate([q[:, :, :, ::-1], q], axis=-1)          # [3,D,N,N]
    psf = np.fft.fftshift(psf, axes=(-2, -1))
    psf = psf / np.sum(psf, axis=(-2, -1), keepdims=True)
    Fpsf = np.fft.rfft2(psf.astype(np.float64)) / float(N * N)    # [3,D,N,WF]
    re = Fpsf.real.astype(np.float32)
    im = Fpsf.imag.astype(np.float32)
    pfc = np.concatenate([re, im, im, -re], axis=-1)              # [3,D,N,4WF]
    return pfc


_PROG_CACHE = {}


def kernel(img, depthmap, heightmap1d, prop_amplitude, prop_phase, H,
           rho_grid, rho_sampling, occlusion):
    occ = bool(np.asarray(occlusion).item())
    img = np.asarray(img, np.float32)
    depthmap = np.asarray(depthmap, np.float32)

    pfc = _host_psf(heightmap1d, prop_amplitude, prop_phase, H,
                    rho_grid, rho_sampling)

    scale = np.float32(img.max())
    imgs = img / scale                                            # [B,C,N,N] f32
    idxf = np.clip(np.floor(depthmap * np.float32(D)), 0, D - 1)[:, 0]  # [B,N,N]
    c1, si, tw1, c2a, c2b, ib = _make_tables()

    if occ not in _PROG_CACHE:
        _PROG_CACHE[occ] = build_program(occ)
    nc = _PROG_CACHE[occ]

    in_maps = []
    for core in range(NCORES):
        b_, c_ = divmod(core, C) if core < B * C else (0, 0)
        in_maps.append({
            "img": np.ascontiguousarray(imgs[b_, c_]),
            "idx": np.ascontiguousarray(idxf[b_]),
            "pfc": np.ascontiguousarray(pfc[c_]),
            "c1": c1, "si": si, "tw1": tw1, "c2a": c2a, "c2b": c2b, "ib": ib,
        })
    t0 = time.perf_counter()
    res_obj = run_bass_kernel_spmd(
        nc, in_maps, list(range(NCORES)),
        trace=bool(os.environ.get("KBASS_TRACE")))
    global LAST
    LAST = {"wall_s": time.perf_counter() - t0,
            "exec_time_ns": res_obj.exec_time_ns,
            "profile_json": res_obj.profile_json}
    res = res_obj.results
    out = np.empty((B, C, N, N), np.float32)
    for core in range(B * C):
        b_, c_ = divmod(core, C)
        out[b_, c_] = res[core]["out"] * scale
    return out
